# revision 11
# baseline (speedup 1.0000x reference)
import numpy as np

import concourse.bacc as bacc
import concourse.mybir as mybir
import concourse.tile as tile
from concourse.bass_utils import run_bass_kernel_spmd

# Problem constants (hardcoded per harness contract)
B, H, W, C = 32, 32, 32, 128
NUM, D0, D1 = 10, 60, 16
JK = D0 * D1            # 960
OO = NUM * JK           # 9600
P = H * W               # 1024 contraction dim of the dense kernel
N_CORES = 8
B_LOC = B // N_CORES    # 4 batches per core
BLK = 480               # dense-kernel column block; 2 blocks = 1 capsule
NBLK = OO // BLK        # 20
EPS = 1e-12

f32 = mybir.dt.float32
f32r = mybir.dt.float32r
bf16 = mybir.dt.bfloat16
AF = mybir.ActivationFunctionType
ALU = mybir.AluOpType


def build_nc():
    nc = bacc.Bacc("TRN2", debug=False)
    u_d = nc.dram_tensor("u", (B_LOC, H, W, C), f32, kind="ExternalInput").ap()
    wc_d = nc.dram_tensor("wc", (2, 2, C, C), f32, kind="ExternalInput").ap()
    km_d = nc.dram_tensor("km", (P, OO), f32, kind="ExternalInput").ap()
    eye_d = nc.dram_tensor("eye", (C, C), f32, kind="ExternalInput").ap()
    out_d = nc.dram_tensor("out", (B_LOC, NUM, JK), f32, kind="ExternalOutput").ap()

    with tile.TileContext(nc) as tc:
        with tc.tile_pool(name="persist", bufs=1) as pers:
            u_hat = pers.tile([128, B_LOC, OO], bf16)       # [n, b, o]
            uT = pers.tile([128, B_LOC, 8, 128], f32r)      # lhsT chunks [p, b, chunk, c]
            wct = pers.tile([128, 4, C], f32r)              # conv taps [ci, tap, co]
            eye = pers.tile([128, C], f32r)
            onesb = pers.tile([128, 128], bf16)
            crep0 = pers.tile([128, 128], bf16)             # uniform c = 0.1
            ones1 = pers.tile([128, 1], f32)                # ss-reduce lhsT
            onesr = pers.tile([1, 128], f32)                # alpha-broadcast lhsT
            c_all = pers.tile([128, B_LOC, NUM], f32)
            z_all = pers.tile([128, B_LOC * NUM], f32)
            cz = pers.tile([128, B_LOC * NUM], f32)
            ss_s = pers.tile([1, B_LOC * NUM], f32)
            blog = pers.tile([128, B_LOC * NUM], f32)
            eexp = pers.tile([128, B_LOC, NUM], f32)
            nmax = pers.tile([128, B_LOC], f32)
            sume = pers.tile([128, B_LOC], f32)
            rsum = pers.tile([128, B_LOC], f32)

            xp = pers.tile([128, B_LOC, 33 * 33], f32r)     # padded conv input per batch
            zcol = pers.tile([128, 33], f32)

            nc.gpsimd.dma_start(wct[:], wc_d.rearrange("dh dw ci co -> ci (dh dw) co"))
            nc.gpsimd.dma_start(eye[:], eye_d)
            nc.vector.memset(onesb[:], 1.0)
            nc.vector.memset(crep0[:], 0.1)
            nc.vector.memset(ones1[:], 1.0)
            nc.vector.memset(onesr[:], 1.0)
            nc.vector.memset(zcol[:], 0.0)

            # ---------- Phase 1: 2x2 SAME conv, pipelined over batches ----------
            # out[co, s=h*32+w] = sum_taps Wtap.T @ xp[b][:, (h+dh)*33 + (w+dw)]
            xp_v = xp[:].rearrange("p b (h w) -> p b h w", w=33)
            with tc.tile_pool(name="convp", bufs=2) as cvp, \
                 tc.tile_pool(name="psc", bufs=2, space="PSUM") as psc, \
                 tc.tile_pool(name="pst", bufs=2, space="PSUM") as pst:
                for b in range(B_LOC):
                    xin = cvp.tile([128, 8, 128], f32r, tag="xin")
                    nc.gpsimd.dma_start(
                        xin[:],
                        u_d[b].rearrange("h w c -> (h w) c").rearrange(
                            "(t sp) c -> sp t c", sp=128))
                    nc.vector.tensor_copy(xp_v[:, b, :, 32], zcol[:])   # right pad col
                    nc.vector.tensor_copy(xp_v[:, b, 32, :], zcol[:])   # bottom pad row
                    for t in range(8):
                        pt = pst.tile([128, 128], f32r, tag="pt")
                        nc.tensor.transpose(pt[:], xin[:, t, :], eye[:])
                        # pt[ch, sp] covers s = t*128 + sp -> rows h = t*4..t*4+4
                        src = pt[:].rearrange("p (a w) -> p a w", w=32)
                        dst = xp_v[:, b, t * 4:(t + 1) * 4, 0:32]
                        if t % 2 == 0:
                            nc.vector.tensor_copy(dst, src)
                        else:
                            nc.scalar.copy(dst, src)
                    for hh in range(2):
                        pc = psc.tile([128, 512], f32, tag="pc")
                        for ti, (dh, dw) in enumerate(((0, 0), (0, 1), (1, 0), (1, 1))):
                            rhs = xp_v[:, b, hh * 16 + dh: hh * 16 + dh + 16, dw:dw + 32]
                            nc.tensor.matmul(pc[:], wct[:, ti, :], rhs,
                                             start=(ti == 0), stop=(ti == 3))
                        # raw-reshape gather: uT[t][pp, c] = conv[a, 8q+t, pp], c = 4a+q
                        pcv = pc[:].rearrange("p (a q t) -> p a q t", q=4, t=8)
                        for t in range(8):
                            src = pcv[:, :, :, t]
                            dst = uT[:, b, t, hh * 64:(hh + 1) * 64].rearrange(
                                "p (a q) -> p a q", q=4)
                            if t % 2 == 0:
                                nc.vector.tensor_copy(dst, src)
                            else:
                                nc.scalar.copy(dst, src)

            # ---------- Phase 2: u_hat = uT.T @ km, with routing iter 0 fused ----------
            # As each capsule's columns complete (2 blocks of 480), emit its
            # iter-0 o-matmuls (uniform c) + z accumulation so they hide here.
            kv = km_d.rearrange("(c p) o -> p c o", p=128)
            with tc.tile_pool(name="kp", bufs=3) as kp, \
                 tc.tile_pool(name="psm", bufs=3, space="PSUM") as psm, \
                 tc.tile_pool(name="psb", bufs=2, space="PSUM") as psb, \
                 tc.tile_pool(name="rt0", bufs=3) as rt0:
                for blk in range(NBLK):
                    kt = kp.tile([128, 8, BLK], f32r, tag="kt")
                    nc.gpsimd.dma_start(kt[:], kv[:, :, blk * BLK:(blk + 1) * BLK])
                    for b in range(B_LOC):
                        pm = psm.tile([128, BLK], f32, tag="pm")
                        for ch in range(8):
                            nc.tensor.matmul(pm[:], uT[:, b, ch, :], kt[:, ch, :],
                                             start=(ch == 0), stop=(ch == 7))
                        dst = u_hat[:, b, blk * BLK:(blk + 1) * BLK]
                        if b % 2 == 0:
                            nc.vector.tensor_copy(dst, pm[:])
                        else:
                            nc.scalar.copy(dst, pm[:])
                    if blk % 2 == 1:
                        i = blk // 2
                        o0 = i * JK
                        for b in range(B_LOC):
                            un = b * NUM + i
                            pbc = psb.tile([128, JK], f32, tag="pbc")
                            nc.tensor.matmul(pbc[:, 0:512], crep0[:],
                                             u_hat[:, b, o0:o0 + 512],
                                             start=True, stop=True)
                            nc.tensor.matmul(pbc[:, 512:JK], crep0[:],
                                             u_hat[:, b, o0 + 512:o0 + JK],
                                             start=True, stop=True)
                            scr = rt0.tile([128, JK], f32, tag="scr")
                            nc.vector.scalar_tensor_tensor(
                                out=scr[:],
                                in0=u_hat[:, b, o0:o0 + JK],
                                scalar=1.0, in1=pbc[:],
                                op0=ALU.mult, op1=ALU.mult,
                                accum_out=z_all[:, un:un + 1])

            # ---------- softmax epilogue (shared) ----------
            def softmax_epilogue(pse, uniform):
                # ss[pair] = sum_c c[c,pair] * z[c,pair]  (== ||o||^2 exactly)
                if uniform:
                    nc.vector.tensor_scalar_mul(cz[:], z_all[:], 0.1)
                else:
                    cf = c_all[:].rearrange("p b i -> p (b i)")
                    nc.vector.tensor_mul(cz[:], cf, z_all[:])
                pss = pse.tile([1, B_LOC * NUM], f32, tag="pss")
                nc.tensor.matmul(pss[:], ones1[:], cz[:], start=True, stop=True)
                nc.vector.tensor_scalar_max(ss_s[:], pss[:], EPS)
                nc.scalar.activation(ss_s[:], ss_s[:], AF.Sqrt)
                nc.vector.reciprocal(ss_s[:], ss_s[:])
                # broadcast alpha to 128 partitions, then b-logits = z * alpha
                pbb = pse.tile([128, B_LOC * NUM], f32, tag="pbb")
                nc.tensor.matmul(pbb[:], onesr[:], ss_s[:], start=True, stop=True)
                nc.vector.tensor_mul(blog[:], z_all[:], pbb[:])
                blv = blog[:].rearrange("p (b i) -> p b i", i=NUM)
                nc.vector.tensor_reduce(nmax[:], blv, axis=mybir.AxisListType.X,
                                        op=ALU.max, negate=True)
                for b in range(B_LOC):
                    nc.scalar.activation(eexp[:, b, :], blv[:, b, :], AF.Exp,
                                         bias=nmax[:, b:b + 1],
                                         accum_out=sume[:, b:b + 1])
                nc.vector.reciprocal(rsum[:], sume[:])
                for b in range(B_LOC):
                    nc.vector.tensor_scalar_mul(
                        c_all[:, b, :], eexp[:, b, :], rsum[:, b:b + 1])

            with tc.tile_pool(name="pse0", bufs=2, space="PSUM") as pse0:
                softmax_epilogue(pse0, uniform=True)

            # ---------- routing iter 1 ----------
            with tc.tile_pool(name="rt", bufs=4) as rt, \
                 tc.tile_pool(name="psb1", bufs=2, space="PSUM") as psb1:
                for i in range(NUM):
                    o0 = i * JK
                    for b in range(B_LOC):
                        un = b * NUM + i
                        crep = rt.tile([128, 128], bf16, tag="crep")
                        nc.scalar.activation(crep[:], onesb[:], AF.Copy,
                                             scale=c_all[:, b, i:i + 1])
                        pbc = psb1.tile([128, JK], f32, tag="pbc")
                        nc.tensor.matmul(pbc[:, 0:512], crep[:],
                                         u_hat[:, b, o0:o0 + 512],
                                         start=True, stop=True)
                        nc.tensor.matmul(pbc[:, 512:JK], crep[:],
                                         u_hat[:, b, o0 + 512:o0 + JK],
                                         start=True, stop=True)
                        scr = rt.tile([128, JK], f32, tag="scr")
                        eng = nc.vector
                        eng.scalar_tensor_tensor(
                            out=scr[:],
                            in0=u_hat[:, b, o0:o0 + JK],
                            scalar=1.0, in1=pbc[:],
                            op0=ALU.mult, op1=ALU.mult,
                            accum_out=z_all[:, un:un + 1])

                with tc.tile_pool(name="pse1", bufs=2, space="PSUM") as pse1:
                    softmax_epilogue(pse1, uniform=False)

                # ---------- routing iter 2: final o, row 0 only, DMA from PSUM ----------
                for i in range(NUM):
                    o0 = i * JK
                    for b in range(B_LOC):
                        ccol = rt.tile([128, 1], bf16, tag="ccol")
                        nc.scalar.copy(ccol[:], c_all[:, b, i:i + 1])
                        pfin = psb1.tile([128, JK], f32, tag="pbc")
                        nc.tensor.matmul(pfin[0:1, 0:512], ccol[:],
                                         u_hat[:, b, o0:o0 + 512],
                                         start=True, stop=True)
                        nc.tensor.matmul(pfin[0:1, 512:JK], ccol[:],
                                         u_hat[:, b, o0 + 512:o0 + JK],
                                         start=True, stop=True)
                        ofin = rt.tile([1, JK], f32, tag="ofin")
                        un = b * NUM + i
                        if un % 2 == 0:
                            nc.vector.tensor_copy(ofin[:], pfin[0:1, :])
                        else:
                            nc.scalar.copy(ofin[:], pfin[0:1, :])
                        nc.sync.dma_start(out_d[b, i], ofin[:])
    nc.compile()
    return nc


_NC_CACHE = None


def _get_nc():
    global _NC_CACHE
    if _NC_CACHE is None:
        _NC_CACHE = build_nc()
    return _NC_CACHE


def kernel(u_vecs, W_conv, kernel):
    u_vecs = np.ascontiguousarray(np.asarray(u_vecs, dtype=np.float32))
    W_conv = np.ascontiguousarray(np.asarray(W_conv, dtype=np.float32))
    km = np.ascontiguousarray(np.asarray(kernel, dtype=np.float32))
    eye = np.eye(C, dtype=np.float32)
    nc = _get_nc()
    in_maps = [
        {"u": u_vecs[ci * B_LOC:(ci + 1) * B_LOC], "wc": W_conv, "km": km, "eye": eye}
        for ci in range(N_CORES)
    ]
    res = run_bass_kernel_spmd(nc, in_maps, core_ids=list(range(N_CORES)))
    out = np.concatenate([r["out"] for r in res.results], axis=0)
    return out.reshape(B, NUM, D0, D1).astype(np.float32)
